# revision 44
# baseline (speedup 1.0000x reference)
"""LiquidNN Trainium2 kernel — layer-pipelined with K-step blocking.

Strategy: 4-stage layer pipeline across cores (one layer per core, weights
SBUF-resident in bf16; cores 4-7 run a redundant second pipeline for SPMD
symmetry).  The time recurrence of ONE layer is local to its core, so each
core advances its layer K consecutive time steps per iteration and ships the
hidden vectors to the next stage in a few AllGathers per iteration (the
block is sliced in thirds so each slice's collective hides under the
remaining slices' compute).  This amortizes the ~10-35 us ncfw collective
cost over K steps instead of paying it every step.

Per-core behavior is injected purely via input data: per-core weights and
per-core gather row-indices (an accumulating indirect DMA pulls the
previous rank's slice out of the gathered arena straight on top of the x
stream; rank 0's indices point at a permanent zero region and its x stream
is real), plus a per-iteration warmup mask folded into tanh's scale.

Critical-path details:
  - the per-step bias add runs as a K=8 matmul into PSUM (start=True seeds
    the bank with the bias), keeping DVE off the tanh chain;
  - each step uses TWO PSUM banks (output features 0-511 / 512-1023) and
    two tanh calls into separate hA/hB tiles, so the next step's W_h
    matmuls on the first feature half only wait for the first tanh;
  - within a step, W_in matmuls (which need only the gathered xin) are
    issued before W_h matmuls (which need the previous step's tanh),
    except on a block's first step where xin is the late input.

Layouts (feature-partitioned, batch on the free axis):
  h/x step tile [128, 256]: tile[p, 32*c + j] = value[feature 128*c + p, batch j]
  W SBUF [128, 8192]:  w[p, 1024*kc + m] = W[128*kc + p, m]
  out = lhsT.T @ rhs accumulated over kc into PSUM [128, 32] per mc chunk.
"""

import numpy as np
import ml_dtypes

B, T, D, H, L = 32, 512, 1024, 1024, 4
N_RANKS = 4
KB = 16  # time steps per pipeline block
N_BLOCKS = T // KB + (N_RANKS - 1)  # pipeline skew in blocks
# AllGather granularity within a block.  The cc stream is the saturated
# resource (AG(S steps) ~ 11.5 + 3.75*S us vs ~4.3 us/step of PE work); a
# short first slice ships early so the next iteration's first steps have
# their input ready, and the last slice's AG (landing ~30 us into the next
# iteration) is only needed 10 steps in.
SLICES = [(0, 8), (8, 16)]

# dtype knob for weights/activations in the recurrence
DT_NP = ml_dtypes.bfloat16

_CACHE = {}


def _w_layout(w, dt):
    # [K, M] -> SBUF [128, 8*1024]
    k, m = w.shape
    assert k == 1024 and m == 1024
    return np.ascontiguousarray(
        w.reshape(8, 128, 1024).transpose(1, 0, 2).reshape(128, 8192)
    ).astype(dt)


def _b_tile(v):
    # [1024] -> [128, 256] chunked per mc
    bb = v.reshape(8, 128).T  # [p, mc]
    return np.ascontiguousarray(
        np.broadcast_to(bb[:, :, None], (128, 8, 32)).reshape(128, 256)
    ).astype(np.float32)


def _x_stream(x, n_blocks, kb, dt):
    # x [B, T, D] -> [n_blocks, KB, 128, 256]
    t = x.shape[1]
    xs = x.transpose(1, 2, 0).reshape(t, 8, 128, B).transpose(0, 2, 1, 3)
    xs = np.ascontiguousarray(xs.reshape(t, 128, 8 * B)).astype(dt)
    n_iter = n_blocks * kb
    if n_iter > t:
        xs = np.concatenate(
            [xs, np.zeros((n_iter - t, 128, 8 * B), dtype=dt)], axis=0
        )
    return np.ascontiguousarray(xs.reshape(n_blocks, kb, 128, 8 * B))


def _build(n_blocks, kb, slices, dt_np, with_tau, skip_cc=False):
    import concourse.bass as bass
    import concourse.bacc as bacc
    import concourse.mybir as mybir
    import concourse.tile as tile

    dt_w = mybir.dt.from_np(np.dtype(dt_np))
    f32 = mybir.dt.float32
    i32 = mybir.dt.int32
    i8 = mybir.dt.int8
    n_sl = len(slices)

    nc = bacc.Bacc(
        "TRN2", target_bir_lowering=False, debug=False, num_devices=8
    )

    wi_d = nc.dram_tensor("wi", [128, 8192], dt_w, kind="ExternalInput")
    wh_d = nc.dram_tensor("wh", [128, 8192], dt_w, kind="ExternalInput")
    wo_d = nc.dram_tensor("wo", [128, 8192], dt_w, kind="ExternalInput")
    bq_d = nc.dram_tensor("bq", [8, 128], dt_w, kind="ExternalInput")
    oh_d = nc.dram_tensor("oh", [8, 256], dt_w, kind="ExternalInput")
    bo_d = nc.dram_tensor("bo", [128, 256], f32, kind="ExternalInput")
    xs_d = nc.dram_tensor("xs", [n_blocks, kb, 128, 256], dt_w, kind="ExternalInput")
    mk_d = nc.dram_tensor("mk", [128, n_blocks], f32, kind="ExternalInput")
    ix_d = nc.dram_tensor("ix", [128, 1], i32, kind="ExternalInput")
    it_d = nc.dram_tensor("it", [128, 256], f32, kind="ExternalInput")
    ic_d = nc.dram_tensor("ic", [128, 256], f32, kind="ExternalInput")
    out_d = nc.dram_tensor("out", [128, 256], f32, kind="ExternalOutput")

    with tile.TileContext(nc) as tc:
        with (
            tc.tile_pool(name="wpool", bufs=1) as wpool,
            tc.tile_pool(name="spool", bufs=2) as spool,
            tc.tile_pool(name="hpool", bufs=2) as hpool,
            tc.tile_pool(name="pspool", bufs=2, space="PSUM") as pspool,
            tc.tile_pool(name="dpool", bufs=2, space="DRAM") as dpool,
        ):
            wi = wpool.tile([128, 8192], dt_w, tag="wi")
            nc.sync.dma_start(wi[:], wi_d.ap())
            wh = wpool.tile([128, 8192], dt_w, tag="wh")
            nc.sync.dma_start(wh[:], wh_d.ap())
            wo = wpool.tile([128, 8192], dt_w, tag="wo")
            nc.sync.dma_start(wo[:], wo_d.ap())
            bq = wpool.tile([8, 128], dt_w, tag="bq")
            nc.sync.dma_start(bq[:], bq_d.ap())
            oh = wpool.tile([8, 256], dt_w, tag="oh")
            nc.sync.dma_start(oh[:], oh_d.ap())
            bo = wpool.tile([128, 256], f32, tag="bo")
            nc.sync.dma_start(bo[:], bo_d.ap())
            ix = wpool.tile([128, 1], i32, tag="ix")
            nc.sync.dma_start(ix[:], ix_d.ap())
            mk_sb = wpool.tile([128, n_blocks], f32, tag="mk")
            nc.sync.dma_start(mk_sb[:], mk_d.ap())
            if with_tau:
                it_t = wpool.tile([128, 256], f32, tag="it")
                nc.sync.dma_start(it_t[:], it_d.ap())
                ic_t = wpool.tile([128, 256], f32, tag="ic")
                nc.sync.dma_start(ic_t[:], ic_d.ap())

            # AllGather arenas, one per slice: rows 0-511 = 4 ranks' gathered
            # h; rows 512-639 = permanent zero region for rank 0's gathers.
            # h is tanh output in [-1,1], so it ships as int8 (x127) — half
            # the collective wire bytes, which is the saturated resource.
            arenas = []
            zt = wpool.tile([128, 256 * max(b - a for a, b in slices)], i8, tag="zt")
            nc.vector.memset(zt[:], 0.0)
            for s, (a, b) in enumerate(slices):
                ws = 256 * (b - a)
                ar = dpool.tile([640, ws], i8, tag=f"arena{s}", name=f"arena{s}")
                for k in range(5):
                    nc.sync.dma_start(ar[128 * k : 128 * (k + 1), :], zt[:, 0:ws])
                arenas.append(ar)

            h0 = hpool.tile([128, 256], dt_w, tag="h_init")
            nc.vector.memset(h0[:], 0.0)
            hA, hB = h0[:, 0:128], h0[:, 128:256]

            mult, add = mybir.AluOpType.mult, mybir.AluOpType.add
            tanh = mybir.ActivationFunctionType.Tanh

            for n in range(n_blocks):
                # Per slice: x half-block (real on rank 0, zeros elsewhere),
                # with the predecessor's h gathered on top by an accumulating
                # indirect DMA (per-core row indices select the prev rank's
                # 128-row block of the arena; rank 0 hits the zero region).
                xins = []
                for s, (a, b) in enumerate(slices):
                    ns = b - a
                    xt = spool.tile([128, ns, 256], dt_w, tag=f"xt{s}", name=f"xt{s}")
                    nc.sync.dma_start(
                        xt[:], xs_d.ap()[n][a:b].transpose([1, 0, 2])
                    )
                    xg = spool.tile([128, 256 * ns], i8, tag=f"xg{s}", name=f"xg{s}")
                    nc.gpsimd.indirect_dma_start(
                        out=xg[:],
                        out_offset=None,
                        in_=arenas[s][:],
                        in_offset=bass.IndirectOffsetOnAxis(
                            ap=ix[:, 0:1], axis=0
                        ),
                    )
                    # xin = dequantized gathered h + x stream (rank 0 gathers
                    # zeros and has the real x; other ranks have zero x)
                    xin = spool.tile([128, 256 * ns], dt_w, tag=f"xin{s}", name=f"xin{s}")
                    nc.vector.scalar_tensor_tensor(
                        xin[:],
                        xg[:],
                        1.0 / 127.0,
                        xt[:].rearrange("p k f -> p (k f)"),
                        mult,
                        add,
                    )
                    xins.append(xin[:])

                for s, (a, b) in enumerate(slices):
                    ship = n < n_blocks - 1  # last block has no consumer
                    agin_sb = spool.tile(
                        [128, 256 * (b - a)],
                        i8,
                        tag=f"agin{s}",
                        name=f"agin{s}",
                    )
                    for j in range(a, b):
                        l = j - a  # step within slice
                        psA = pspool.tile([128, 128], f32, tag="psA", name="psA")
                        psB = pspool.tile([128, 128], f32, tag="psB", name="psB")
                        # bias seeds both banks (start=True marks the whole
                        # 128-col range written, so everything accumulates)
                        nc.tensor.matmul(
                            psA[:], bq[:], oh[:, 0:128],
                            start=True, stop=False, skip_group_check=True,
                        )
                        nc.tensor.matmul(
                            psB[:], bq[:], oh[:, 128:256],
                            start=True, stop=False, skip_group_check=True,
                        )
                        wh_ops = [
                            (wh, (hA if kc < 4 else hB), 32 * (kc % 4), kc)
                            for kc in range(8)
                        ]
                        wi_ops = [
                            (wi, xins[s], 256 * l + 32 * kc, kc)
                            for kc in range(8)
                        ]
                        ops = wh_ops + wi_ops if j == a else wi_ops + wh_ops
                        hAn = hpool.tile([128, 128], dt_w, tag="hA", name="hA")
                        hBn = hpool.tile([128, 128], dt_w, tag="hB", name="hB")
                        for mc in range(8):
                            psX = psA if mc < 4 else psB
                            pslice = psX[:, 32 * (mc % 4) : 32 * (mc % 4) + 32]
                            for i, (w, r, off, kc) in enumerate(ops):
                                nc.tensor.matmul(
                                    pslice,
                                    w[:, 1024 * kc + 128 * mc : 1024 * kc + 128 * mc + 128],
                                    r[:, off : off + 32],
                                    start=False,
                                    stop=(i == 15 and mc in (3, 7)),
                                    skip_group_check=True,
                                )
                            if mc == 3 or mc == 7:
                                lohalf = mc == 3
                                psX_, hN = (psA, hAn) if lohalf else (psB, hBn)
                                if with_tau:
                                    dx = hpool.tile(
                                        [128, 128], dt_w, tag="dx", name="dx"
                                    )
                                    nc.scalar.activation(
                                        dx[:], psX_[:], tanh,
                                        scale=mk_sb[:, n : n + 1],
                                    )
                                    hm = hpool.tile(
                                        [128, 128], f32, tag="hm", name="hm"
                                    )
                                    icx = ic_t[:, 0:128] if lohalf else ic_t[:, 128:256]
                                    nc.vector.tensor_mul(
                                        hm[:], (hA if lohalf else hB), icx
                                    )
                                    nc.vector.scalar_tensor_tensor(
                                        hN[:], dx[:], 1.0, hm[:], mult, add
                                    )
                                else:
                                    nc.scalar.activation(
                                        hN[:], psX_[:], tanh,
                                        scale=mk_sb[:, n : n + 1],
                                    )
                        # stage h into the ship buffer, quantizing to int8
                        # (off the tanh->W_h critical chain; DVE is idle)
                        nc.vector.tensor_scalar_mul(
                            agin_sb[:, 256 * l : 256 * l + 128], hAn[:], 127.0
                        )
                        nc.vector.tensor_scalar_mul(
                            agin_sb[:, 256 * l + 128 : 256 * l + 256], hBn[:], 127.0
                        )
                        hA, hB = hAn[:], hBn[:]

                    # slice complete: ship it
                    if ship:
                        agin = dpool.tile(
                            [128, 256 * (b - a)],
                            i8,
                            tag=f"agin_d{s}",
                            name=f"agin_d{s}",
                        )
                        nc.sync.dma_start(agin[:], agin_sb[:])
                    if ship and not skip_cc:
                        nc.gpsimd.collective_compute(
                            "AllGather",
                            mybir.AluOpType.bypass,
                            ins=[agin[:]],
                            outs=[arenas[s][0:512, :]],
                            replica_groups=[[0, 1, 2, 3], [4, 5, 6, 7]],
                        )

            ps2 = pspool.tile([128, 256], f32, tag="ps2")
            for mc in range(8):
                pslice = ps2[:, 32 * mc : 32 * mc + 32]
                for kc in range(8):
                    nc.tensor.matmul(
                        pslice,
                        wo[:, 1024 * kc + 128 * mc : 1024 * kc + 128 * mc + 128],
                        (hA if kc < 4 else hB)[:, 32 * (kc % 4) : 32 * (kc % 4) + 32],
                        start=(kc == 0),
                        stop=(kc == 7),
                    )
            nc.vector.tensor_add(ps2[:], ps2[:], bo[:])
            osb = spool.tile([128, 256], f32, tag="osb")
            nc.vector.tensor_copy(osb[:], ps2[:])
            nc.sync.dma_start(out_d.ap(), osb[:])

    nc.compile()
    return nc


def _in_maps(x, W_in, b_in, W_h, b_h, tau, W_out, b_out, n_blocks, kb, dt_np):
    xs_real = _x_stream(np.asarray(x), n_blocks, kb, dt_np)
    xs_zero = np.zeros_like(xs_real)
    wo = _w_layout(np.asarray(W_out), dt_np)
    bo = _b_tile(np.asarray(b_out))
    oh = np.zeros((8, 256), dtype=dt_np)
    for k in range(8):
        oh[k, 32 * k : 32 * (k + 1)] = 1.0
    maps = []
    for c in range(8):
        r = c % 4
        base = 128 * (r - 1) if r > 0 else 512
        ix = (base + np.arange(128, dtype=np.int32)).reshape(128, 1)
        mk = np.zeros((128, n_blocks), dtype=np.float32)
        mk[:, r : r + T // kb] = 1.0
        bias = (np.asarray(b_in[r]) + np.asarray(b_h[r])).astype(np.float64)
        m = {
            "wi": _w_layout(np.asarray(W_in[r]), dt_np),
            "wh": _w_layout(np.asarray(W_h[r]), dt_np),
            "wo": wo,
            "bq": np.ascontiguousarray(bias.reshape(8, 128)).astype(dt_np),
            "oh": oh,
            "bo": bo,
            "xs": xs_real if r == 0 else xs_zero,
            "mk": mk,
            "ix": ix,
            "it": _b_tile(1.0 / np.asarray(tau[r], dtype=np.float64)),
            "ic": _b_tile(1.0 - 1.0 / np.asarray(tau[r], dtype=np.float64)),
        }
        maps.append(m)
    return maps


def _unshard_out(res):
    # [128, 256] -> [32, 1024]
    return np.ascontiguousarray(
        res.reshape(128, 8, 32).transpose(2, 1, 0).reshape(32, 1024)
    ).astype(np.float32)


def run_hw(x, W_in, b_in, W_h, b_h, tau, W_out, b_out, trace=False):
    from concourse import bass_utils

    with_tau = not np.allclose(np.asarray(tau), 1.0)
    key = (N_BLOCKS, KB, np.dtype(DT_NP).name, with_tau)
    if key not in _CACHE:
        _CACHE[key] = _build(N_BLOCKS, KB, SLICES, DT_NP, with_tau)
    nc = _CACHE[key]
    maps = _in_maps(
        x, W_in, b_in, W_h, b_h, tau, W_out, b_out, N_BLOCKS, KB, DT_NP
    )
    res = bass_utils.run_bass_kernel_spmd(
        nc, maps, core_ids=list(range(8)), trace=trace
    )
    out = _unshard_out(res.results[3]["out"])
    return out, res


def kernel(x, W_in, b_in, W_h, b_h, tau, W_out, b_out):
    out, _ = run_hw(x, W_in, b_in, W_h, b_h, tau, W_out, b_out)
    return out


# revision 53
# speedup vs baseline: 1.0340x; 1.0340x over previous
"""LiquidNN Trainium2 kernel — layer-pipelined with K-step blocking.

Strategy: 4-stage layer pipeline across cores (one layer per core, weights
SBUF-resident in bf16; cores 4-7 run a redundant second pipeline for SPMD
symmetry).  The time recurrence of ONE layer is local to its core, so each
core advances its layer K consecutive time steps per iteration and ships the
hidden vectors to the next stage in a few AllGathers per iteration (the
block is sliced in thirds so each slice's collective hides under the
remaining slices' compute).  This amortizes the ~10-35 us ncfw collective
cost over K steps instead of paying it every step.

Per-core behavior is injected purely via input data: per-core weights and
per-core gather row-indices (an accumulating indirect DMA pulls the
previous rank's slice out of the gathered arena straight on top of the x
stream; rank 0's indices point at a permanent zero region and its x stream
is real), plus a per-iteration warmup mask folded into tanh's scale.

Critical-path details:
  - the per-step bias add runs as a K=8 matmul into PSUM (start=True seeds
    the bank with the bias), keeping DVE off the tanh chain;
  - each step uses TWO PSUM banks (output features 0-511 / 512-1023) and
    two tanh calls into separate hA/hB tiles, so the next step's W_h
    matmuls on the first feature half only wait for the first tanh;
  - within a step, W_in matmuls (which need only the gathered xin) are
    issued before W_h matmuls (which need the previous step's tanh),
    except on a block's first step where xin is the late input.

Layouts (feature-partitioned, batch on the free axis):
  h/x step tile [128, 256]: tile[p, 32*c + j] = value[feature 128*c + p, batch j]
  W SBUF [128, 8192]:  w[p, 1024*kc + m] = W[128*kc + p, m]
  out = lhsT.T @ rhs accumulated over kc into PSUM [128, 32] per mc chunk.
"""

import numpy as np
import ml_dtypes

B, T, D, H, L = 32, 512, 1024, 1024, 4
N_RANKS = 4
KB = 16  # time steps per pipeline block
N_BLOCKS = T // KB + (N_RANKS - 1)  # pipeline skew in blocks
# AllGather granularity within a block.  The cc stream is the saturated
# resource (AG(S steps) ~ 11.5 + 3.75*S us vs ~4.3 us/step of PE work); a
# short first slice ships early so the next iteration's first steps have
# their input ready, and the last slice's AG (landing ~30 us into the next
# iteration) is only needed 10 steps in.
SLICES = [(0, 8), (8, 16)]

# dtype knob for weights/activations in the recurrence
DT_NP = ml_dtypes.bfloat16

_CACHE = {}


def _w_layout(w, dt):
    # [K, M] -> SBUF [128, 8*1024]
    k, m = w.shape
    assert k == 1024 and m == 1024
    return np.ascontiguousarray(
        w.reshape(8, 128, 1024).transpose(1, 0, 2).reshape(128, 8192)
    ).astype(dt)


def _b_tile(v):
    # [1024] -> [128, 256] chunked per mc
    bb = v.reshape(8, 128).T  # [p, mc]
    return np.ascontiguousarray(
        np.broadcast_to(bb[:, :, None], (128, 8, 32)).reshape(128, 256)
    ).astype(np.float32)


def _x_stream(x, n_blocks, kb, dt):
    # x [B, T, D] -> [n_blocks, KB, 128, 256]
    t = x.shape[1]
    xs = x.transpose(1, 2, 0).reshape(t, 8, 128, B).transpose(0, 2, 1, 3)
    xs = np.ascontiguousarray(xs.reshape(t, 128, 8 * B)).astype(dt)
    n_iter = n_blocks * kb
    if n_iter > t:
        xs = np.concatenate(
            [xs, np.zeros((n_iter - t, 128, 8 * B), dtype=dt)], axis=0
        )
    return np.ascontiguousarray(xs.reshape(n_blocks, kb, 128, 8 * B))


def _build(n_blocks, kb, slices, dt_np, with_tau, skip_cc=False):
    import concourse.bass as bass
    import concourse.bacc as bacc
    import concourse.mybir as mybir
    import concourse.tile as tile

    dt_w = mybir.dt.from_np(np.dtype(dt_np))
    f32 = mybir.dt.float32
    i32 = mybir.dt.int32
    i8 = mybir.dt.int8
    n_sl = len(slices)

    nc = bacc.Bacc(
        "TRN2", target_bir_lowering=False, debug=False, num_devices=8
    )

    wi_d = nc.dram_tensor("wi", [128, 8192], dt_w, kind="ExternalInput")
    wh_d = nc.dram_tensor("wh", [128, 8192], dt_w, kind="ExternalInput")
    wo_d = nc.dram_tensor("wo", [128, 8192], dt_w, kind="ExternalInput")
    bq_d = nc.dram_tensor("bq", [8, 128], dt_w, kind="ExternalInput")
    oh_d = nc.dram_tensor("oh", [8, 256], dt_w, kind="ExternalInput")
    bo_d = nc.dram_tensor("bo", [128, 256], f32, kind="ExternalInput")
    xs_d = nc.dram_tensor("xs", [n_blocks, kb, 128, 256], dt_w, kind="ExternalInput")
    mk_d = nc.dram_tensor("mk", [128, n_blocks], f32, kind="ExternalInput")
    ix_d = nc.dram_tensor("ix", [128, 1], i32, kind="ExternalInput")
    it_d = nc.dram_tensor("it", [128, 256], f32, kind="ExternalInput")
    ic_d = nc.dram_tensor("ic", [128, 256], f32, kind="ExternalInput")
    out_d = nc.dram_tensor("out", [128, 256], f32, kind="ExternalOutput")

    with tile.TileContext(nc) as tc:
        with (
            tc.tile_pool(name="wpool", bufs=1) as wpool,
            tc.tile_pool(name="spool", bufs=3) as spool,
            # 4 h slots: the quantize reader on the in-order DVE queue can
            # lag a few steps behind PE (stuck behind a gather-waiting
            # dequant), and 2 slots would stall PE on tile recycling
            tc.tile_pool(name="hpool", bufs=4) as hpool,
            tc.tile_pool(name="pspool", bufs=3, space="PSUM") as pspool,
            tc.tile_pool(name="dpool", bufs=3, space="DRAM") as dpool,
        ):
            wi = wpool.tile([128, 8192], dt_w, tag="wi")
            nc.sync.dma_start(wi[:], wi_d.ap())
            wh = wpool.tile([128, 8192], dt_w, tag="wh")
            nc.sync.dma_start(wh[:], wh_d.ap())
            wo = wpool.tile([128, 8192], dt_w, tag="wo")
            nc.sync.dma_start(wo[:], wo_d.ap())
            bq = wpool.tile([8, 128], dt_w, tag="bq")
            nc.sync.dma_start(bq[:], bq_d.ap())
            oh = wpool.tile([8, 256], dt_w, tag="oh")
            nc.sync.dma_start(oh[:], oh_d.ap())
            bo = wpool.tile([128, 256], f32, tag="bo")
            nc.sync.dma_start(bo[:], bo_d.ap())
            ix = wpool.tile([128, 1], i32, tag="ix")
            nc.sync.dma_start(ix[:], ix_d.ap())
            mk_sb = wpool.tile([128, n_blocks], f32, tag="mk")
            nc.sync.dma_start(mk_sb[:], mk_d.ap())
            if with_tau:
                it_t = wpool.tile([128, 256], f32, tag="it")
                nc.sync.dma_start(it_t[:], it_d.ap())
                ic_t = wpool.tile([128, 256], f32, tag="ic")
                nc.sync.dma_start(ic_t[:], ic_d.ap())

            # AllGather arenas, one per slice: rows 0-511 = 4 ranks' gathered
            # h; rows 512-639 = permanent zero region for rank 0's gathers.
            # h is tanh output in [-1,1], so it ships as int8 (x127) — half
            # the collective wire bytes, which is the saturated resource.
            arenas = []
            zt = wpool.tile([128, 256 * max(b - a for a, b in slices)], i8, tag="zt")
            nc.vector.memset(zt[:], 0.0)
            for s, (a, b) in enumerate(slices):
                ws = 256 * (b - a)
                ar = dpool.tile([640, ws], i8, tag=f"arena{s}", name=f"arena{s}")
                for k in range(5):
                    nc.sync.dma_start(ar[128 * k : 128 * (k + 1), :], zt[:, 0:ws])
                arenas.append(ar)

            h0 = hpool.tile([128, 256], dt_w, tag="h_init")
            nc.vector.memset(h0[:], 0.0)
            hA, hB = h0[:, 0:128], h0[:, 128:256]

            mult, add = mybir.AluOpType.mult, mybir.AluOpType.add
            tanh = mybir.ActivationFunctionType.Tanh

            def emit_fetch(n, s, a, b):
                # Prepare iteration n's slice-s input: x half-block (real on
                # rank 0, zeros elsewhere) with the predecessor's h gathered
                # on top by an indirect DMA (per-core row indices select the
                # prev rank's 128-row arena block; rank 0 hits the zero
                # region), dequantized from int8 in the same DVE op.
                ns = b - a
                xt = spool.tile(
                    [128, ns, 256], dt_w, tag=f"xt{s}", name=f"xt{s}"
                )
                nc.sync.dma_start(
                    xt[:], xs_d.ap()[n][a:b].transpose([1, 0, 2])
                )
                xg = spool.tile(
                    [128, 256 * ns], i8, tag=f"xg{s}", name=f"xg{s}"
                )
                nc.gpsimd.indirect_dma_start(
                    out=xg[:],
                    out_offset=None,
                    in_=arenas[s][:],
                    in_offset=bass.IndirectOffsetOnAxis(ap=ix[:, 0:1], axis=0),
                )
                xin = spool.tile(
                    [128, 256 * ns], dt_w, tag=f"xin{s}", name=f"xin{s}"
                )
                nc.vector.scalar_tensor_tensor(
                    xin[:],
                    xg[:],
                    1.0 / 127.0,
                    xt[:].rearrange("p k f -> p (k f)"),
                    mult,
                    add,
                )
                return xin[:]

            # software-pipelined fetch: slice s of iteration n+1 is emitted
            # right after iteration n's slice-s AllGather trigger, so the
            # gather fires mid-iteration as soon as that AG completes instead
            # of queuing behind the last AG trigger at the boundary
            xins = [emit_fetch(0, s, a, b) for s, (a, b) in enumerate(slices)]

            for n in range(n_blocks):
                xins_cur, xins = xins, [None] * n_sl

                for s, (a, b) in enumerate(slices):
                    ship = n < n_blocks - 1  # last block has no consumer
                    agin_sb = spool.tile(
                        [128, 256 * (b - a)],
                        i8,
                        tag=f"agin{s}",
                        name=f"agin{s}",
                    )
                    for j in range(a, b):
                        l = j - a  # step within slice
                        psA = pspool.tile([128, 128], f32, tag="psA", name="psA")
                        psB = pspool.tile([128, 128], f32, tag="psB", name="psB")
                        # bias seeds both banks (start=True marks the whole
                        # 128-col range written, so everything accumulates)
                        nc.tensor.matmul(
                            psA[:], bq[:], oh[:, 0:128],
                            start=True, stop=False, skip_group_check=True,
                        )
                        nc.tensor.matmul(
                            psB[:], bq[:], oh[:, 128:256],
                            start=True, stop=False, skip_group_check=True,
                        )
                        wh_ops = [
                            (wh, (hA if kc < 4 else hB), 32 * (kc % 4), kc)
                            for kc in range(8)
                        ]
                        wi_ops = [
                            (wi, xins_cur[s], 256 * l + 32 * kc, kc)
                            for kc in range(8)
                        ]
                        ops = wh_ops + wi_ops if j == a else wi_ops + wh_ops
                        hAn = hpool.tile([128, 128], dt_w, tag="hA", name="hA")
                        hBn = hpool.tile([128, 128], dt_w, tag="hB", name="hB")
                        for mc in range(8):
                            psX = psA if mc < 4 else psB
                            pslice = psX[:, 32 * (mc % 4) : 32 * (mc % 4) + 32]
                            for i, (w, r, off, kc) in enumerate(ops):
                                nc.tensor.matmul(
                                    pslice,
                                    w[:, 1024 * kc + 128 * mc : 1024 * kc + 128 * mc + 128],
                                    r[:, off : off + 32],
                                    start=False,
                                    stop=(i == 15 and mc in (3, 7)),
                                    skip_group_check=True,
                                )
                            if mc == 3 or mc == 7:
                                lohalf = mc == 3
                                psX_, hN = (psA, hAn) if lohalf else (psB, hBn)
                                if with_tau:
                                    dx = hpool.tile(
                                        [128, 128], dt_w, tag="dx", name="dx"
                                    )
                                    nc.scalar.activation(
                                        dx[:], psX_[:], tanh,
                                        scale=mk_sb[:, n : n + 1],
                                    )
                                    hm = hpool.tile(
                                        [128, 128], f32, tag="hm", name="hm"
                                    )
                                    icx = ic_t[:, 0:128] if lohalf else ic_t[:, 128:256]
                                    nc.vector.tensor_mul(
                                        hm[:], (hA if lohalf else hB), icx
                                    )
                                    nc.vector.scalar_tensor_tensor(
                                        hN[:], dx[:], 1.0, hm[:], mult, add
                                    )
                                else:
                                    nc.scalar.activation(
                                        hN[:], psX_[:], tanh,
                                        scale=mk_sb[:, n : n + 1],
                                    )
                        # stage h into the ship buffer, quantizing to int8
                        # (off the tanh->W_h critical chain; DVE is idle)
                        nc.vector.tensor_scalar_mul(
                            agin_sb[:, 256 * l : 256 * l + 128], hAn[:], 127.0
                        )
                        nc.vector.tensor_scalar_mul(
                            agin_sb[:, 256 * l + 128 : 256 * l + 256], hBn[:], 127.0
                        )
                        hA, hB = hAn[:], hBn[:]

                    # slice complete: ship it
                    if ship:
                        agin = dpool.tile(
                            [128, 256 * (b - a)],
                            i8,
                            tag=f"agin_d{s}",
                            name=f"agin_d{s}",
                        )
                        nc.sync.dma_start(agin[:], agin_sb[:])
                    if ship and not skip_cc:
                        nc.gpsimd.collective_compute(
                            "AllGather",
                            mybir.AluOpType.bypass,
                            ins=[agin[:]],
                            outs=[arenas[s][0:512, :]],
                            replica_groups=[[0, 1, 2, 3], [4, 5, 6, 7]],
                        )
                    if ship:
                        xins[s] = emit_fetch(n + 1, s, a, b)

            ps2 = pspool.tile([128, 256], f32, tag="ps2", bufs=1)
            for mc in range(8):
                pslice = ps2[:, 32 * mc : 32 * mc + 32]
                for kc in range(8):
                    nc.tensor.matmul(
                        pslice,
                        wo[:, 1024 * kc + 128 * mc : 1024 * kc + 128 * mc + 128],
                        (hA if kc < 4 else hB)[:, 32 * (kc % 4) : 32 * (kc % 4) + 32],
                        start=(kc == 0),
                        stop=(kc == 7),
                    )
            nc.vector.tensor_add(ps2[:], ps2[:], bo[:])
            osb = spool.tile([128, 256], f32, tag="osb")
            nc.vector.tensor_copy(osb[:], ps2[:])
            nc.sync.dma_start(out_d.ap(), osb[:])

    nc.compile()
    return nc


def _in_maps(x, W_in, b_in, W_h, b_h, tau, W_out, b_out, n_blocks, kb, dt_np):
    xs_real = _x_stream(np.asarray(x), n_blocks, kb, dt_np)
    xs_zero = np.zeros_like(xs_real)
    wo = _w_layout(np.asarray(W_out), dt_np)
    bo = _b_tile(np.asarray(b_out))
    oh = np.zeros((8, 256), dtype=dt_np)
    for k in range(8):
        oh[k, 32 * k : 32 * (k + 1)] = 1.0
    maps = []
    for c in range(8):
        r = c % 4
        base = 128 * (r - 1) if r > 0 else 512
        ix = (base + np.arange(128, dtype=np.int32)).reshape(128, 1)
        mk = np.zeros((128, n_blocks), dtype=np.float32)
        mk[:, r : r + T // kb] = 1.0
        bias = (np.asarray(b_in[r]) + np.asarray(b_h[r])).astype(np.float64)
        m = {
            "wi": _w_layout(np.asarray(W_in[r]), dt_np),
            "wh": _w_layout(np.asarray(W_h[r]), dt_np),
            "wo": wo,
            "bq": np.ascontiguousarray(bias.reshape(8, 128)).astype(dt_np),
            "oh": oh,
            "bo": bo,
            "xs": xs_real if r == 0 else xs_zero,
            "mk": mk,
            "ix": ix,
            "it": _b_tile(1.0 / np.asarray(tau[r], dtype=np.float64)),
            "ic": _b_tile(1.0 - 1.0 / np.asarray(tau[r], dtype=np.float64)),
        }
        maps.append(m)
    return maps


def _unshard_out(res):
    # [128, 256] -> [32, 1024]
    return np.ascontiguousarray(
        res.reshape(128, 8, 32).transpose(2, 1, 0).reshape(32, 1024)
    ).astype(np.float32)


def run_hw(x, W_in, b_in, W_h, b_h, tau, W_out, b_out, trace=False):
    from concourse import bass_utils

    with_tau = not np.allclose(np.asarray(tau), 1.0)
    key = (N_BLOCKS, KB, np.dtype(DT_NP).name, with_tau)
    if key not in _CACHE:
        _CACHE[key] = _build(N_BLOCKS, KB, SLICES, DT_NP, with_tau)
    nc = _CACHE[key]
    maps = _in_maps(
        x, W_in, b_in, W_h, b_h, tau, W_out, b_out, N_BLOCKS, KB, DT_NP
    )
    res = bass_utils.run_bass_kernel_spmd(
        nc, maps, core_ids=list(range(8)), trace=trace
    )
    out = _unshard_out(res.results[3]["out"])
    return out, res


def kernel(x, W_in, b_in, W_h, b_h, tau, W_out, b_out):
    out, _ = run_hw(x, W_in, b_in, W_h, b_h, tau, W_out, b_out)
    return out
